# revision 1
# baseline (speedup 1.0000x reference)
"""Trainium2 Bass kernel for DiffusionGraphConv (DCRNN-style graph diffusion).

Math (per reference):
  x0 = reshape(inputs) -> [N, P*B]
  for each of 2 sparse transition matrices A (COO, E edges):
     x1 = A @ x0 ;  x2 = 2*A@x1 - x0
  out = concat([x0, x1_a, x2_a, x1_b, x2_b]) @ weight + bias

Strategy:
  - Data-parallel over batch: each of 8 cores takes 4 batches -> feature
    width F = 4*32 = 128 per core; no collectives.
  - SpMM via: dma_gather (edge-sorted row gather from HBM, fp16, 256B rows)
    followed by compressed one-hot scatter matmuls on the TensorEngine
    (val folded into the one-hot in fp16, fp32 PSUM accumulation).
  - Edges sorted by (dst, src), padded per 128-row node tile to multiples of
    128; each chunk of 128 edges becomes one matmul whose stationary operand
    is S[e, dst_local - o_c] = val_e over the chunk's dst window.
  - x2 terms are never materialized: out = x0@(W0-W2-W4) + x1a@W1
    + (A x1a)@(2W2) + x1b@W3 + (B x1b)@(2W4), with the W's folded host-side.
  - fp16 data path everywhere, fp32 accumulation (PSUM + output accumulator):
    measured end-to-end relative error ~5e-4.
"""

import sys

import numpy as np

sys.path.insert(0, "/opt/trn_rl_repo")

import concourse.bass as bass
import concourse.bacc as bacc
import concourse.mybir as mybir
import concourse.tile as tile
from concourse.bass_utils import run_bass_kernel_spmd

dt = mybir.dt

N, P, Q, B, E = 10000, 32, 64, 32, 160000
NT = 79              # 128-row node tiles
NPAD = NT * 128      # 10112
F = 128              # features per core: 4 batches x 32
FO = 256             # output features per core: 4 batches x 64
NCORES = 8
GROUP = 4            # node tiles per dma_gather call


def _prep_graph(idx, val):
    """Preprocess one sparse matrix into the device schedule.

    Returns dict with:
      idx_w [128, Lt] int16 : gather indices, wrapped by 16, rows tiled to 128
      S     [128, Ct] fp16  : per-chunk one-hot scatter blocks (val folded)
      tiles : per node tile, list of chunks (M_c, o_c, q_c)
      tile_ioff : per tile, logical start offset into the idx stream
    """
    dst = idx[0].astype(np.int64)
    src = idx[1].astype(np.int64)
    val = val.astype(np.float32)
    order = np.lexsort((src, dst))
    dst, src, val = dst[order], src[order], val[order]

    srcs_all = []
    S_cols = []
    tiles = []
    tile_ioff = []
    qtot = 0
    ioff = 0
    for i in range(NT):
        lo = i * 128
        # matmul PSUM base partition must be 0/32/64/96 (PE quadrant
        # tiling), so bucket edges by 32-row dst quadrant: each chunk's
        # scatter window then sits inside one quadrant.
        chunks = []
        t_src = []
        for quad in range(4):
            a = np.searchsorted(dst, lo + 32 * quad)
            b = np.searchsorted(dst, lo + 32 * (quad + 1))
            d_l = dst[a:b] - (lo + 32 * quad)     # in [0, 32)
            s_l = src[a:b]
            v_l = val[a:b]
            ne = len(d_l)
            if ne == 0:
                continue
            npad_e = (-ne) % 128
            d_l = np.concatenate([d_l, np.zeros(npad_e, np.int64)])
            s_l = np.concatenate([s_l, np.zeros(npad_e, np.int64)])
            v_l = np.concatenate([v_l, np.zeros(npad_e, np.float32)])
            for c in range(len(d_l) // 128):
                dl = d_l[c * 128:(c + 1) * 128]
                vl = v_l[c * 128:(c + 1) * 128]
                M_c = int(dl.max()) + 1           # <= 32
                S = np.zeros((128, M_c), np.float16)
                S[np.arange(128), dl] = vl.astype(np.float16)
                chunks.append((M_c, 32 * quad, qtot))
                S_cols.append(S)
                qtot += M_c
            t_src.append(s_l)
        if not chunks:                            # tile with no edges at all
            S = np.zeros((128, 1), np.float16)
            chunks.append((1, 0, qtot))
            S_cols.append(S)
            qtot += 1
            t_src.append(np.zeros(128, np.int64))
        tiles.append(chunks)
        tile_ioff.append(ioff)
        s_all = np.concatenate(t_src)
        srcs_all.append(s_all)
        ioff += len(s_all)
    srcs = np.concatenate(srcs_all).astype(np.int16)   # [ioff]
    # wrap by 16: logical i lives at [i % 16, i // 16]; tile rows to 128
    idx_w = np.tile(srcs.reshape(-1, 16).T, (8, 1)).copy()  # [128, ioff//16]
    S = np.concatenate(S_cols, axis=1)                      # [128, qtot]
    return {"idx_w": idx_w, "S": S, "tiles": tiles, "tile_ioff": tile_ioff,
            "total_idx": ioff, "total_q": qtot}


def _make_groups(g):
    """Split the 79 tiles into gather groups of GROUP tiles, and build the
    packed per-group [S | idx] int16 stream (single DMA per group)."""
    groups = []
    packed_cols = []
    pc = 0
    for i0 in range(0, NT, GROUP):
        tl = list(range(i0, min(i0 + GROUP, NT)))
        ioff0 = g["tile_ioff"][tl[0]]
        gch = sum(len(g["tiles"][i]) for i in tl)
        q0 = g["tiles"][tl[0]][0][2]
        qcols = sum(M for i in tl for (M, _, _) in g["tiles"][i])
        ccols = gch * 8                     # 128 int16 idx = 8 cols
        S_blk = g["S"][:, q0:q0 + qcols].view(np.int16)
        idx_blk = g["idx_w"][:, ioff0 // 16: ioff0 // 16 + ccols]
        packed_cols.append(np.concatenate([S_blk, idx_blk], axis=1))
        groups.append({"tiles": tl, "gch": gch, "q0": q0, "qcols": qcols,
                       "ccols": ccols, "p0": pc})
        pc += qcols + ccols
    packed = np.ascontiguousarray(np.concatenate(packed_cols, axis=1))
    return groups, packed


def _build_nc(g1, g2, grp1, grp2, parts=4):
    """parts: 1=x0 proj only, 2=+t0 phase0, 3=+t0 phase1, 4=full."""
    nc = bacc.Bacc("TRN2", target_bir_lowering=False, debug=False,
                   num_devices=NCORES)

    groups1, packed1 = grp1
    groups2, packed2 = grp2
    P1 = packed1.shape[1]
    P2 = packed2.shape[1]

    x0nm = nc.declare_dram_parameter("x0nm", [NPAD, F], dt.float16, isOutput=False)
    x0T_d = nc.declare_dram_parameter("x0T", [F, NPAD], dt.float16, isOutput=False)
    pk_d = [nc.declare_dram_parameter("pk1", [128, P1], dt.int16, isOutput=False),
            nc.declare_dram_parameter("pk2", [128, P2], dt.int16, isOutput=False)]
    W5_d = nc.declare_dram_parameter("W5", [128, 5 * FO], dt.float16, isOutput=False)
    ones_d = nc.declare_dram_parameter("ones_", [1, 128], dt.float16, isOutput=False)
    zrow_d = nc.declare_dram_parameter("zrow", [1, 128], dt.float16, isOutput=False)
    bias_d = nc.declare_dram_parameter("biasrow", [1, FO], dt.float16, isOutput=False)
    ident_d = nc.declare_dram_parameter("ident", [128, 128], dt.float16, isOutput=False)
    out_d = nc.declare_dram_parameter("out", [NPAD, FO], dt.float32,
                                      isOutput=True)
    x1hbm = [nc.dram_tensor("x1hbm_a", [NPAD, F], dt.float16),
             nc.dram_tensor("x1hbm_b", [NPAD, F], dt.float16)]

    graphs = (g1, g2)
    all_groups = (groups1, groups2)
    gch_max = max(gr["gch"] for gg in all_groups for gr in gg)
    pcols_max = 0
    for gg in all_groups:
        for s0 in range(0, len(gg), 4):
            pcols_max = max(pcols_max, sum(
                x["qcols"] + x["ccols"] for x in gg[s0:s0 + 4]))

    with tile.TileContext(nc) as tc:
        with (
            tc.tile_pool(name="const", bufs=1) as constp,
            tc.tile_pool(name="outacc", bufs=1) as outp,
            tc.tile_pool(name="x0t", bufs=1) as x0tp,
            tc.tile_pool(name="gpool", bufs=24) as gp,
            tc.tile_pool(name="pkpool", bufs=3) as pkp,
            tc.tile_pool(name="xt", bufs=4) as xtp,
            tc.tile_pool(name="xT", bufs=4) as xTp,
            tc.tile_pool(name="ypsum", bufs=3, space="PSUM") as yps,
            tc.tile_pool(name="tpsum", bufs=2, space="PSUM") as tps,
            tc.tile_pool(name="ppsum", bufs=2, space="PSUM") as pps,
        ):
            W5t = constp.tile([128, 5 * FO], dt.float16, tag="w5")
            nc.sync.dma_start(W5t[:], W5_d[:, :])
            onescol = constp.tile([1, 128], dt.float16, tag="ones")
            nc.sync.dma_start(onescol[:], ones_d[:, :])
            zrow = constp.tile([1, 128], dt.float16, tag="zrow")
            nc.sync.dma_start(zrow[:], zrow_d[:, :])
            biasrow = constp.tile([1, FO], dt.float16, tag="bias")
            nc.sync.dma_start(biasrow[:], bias_d[:, :])
            ident = constp.tile([128, 128], dt.float16, tag="ident")
            nc.sync.dma_start(ident[:], ident_d[:, :])

            out_acc = outp.tile([128, NT * FO], dt.float32, tag="oacc")

            # registers holding num_idxs for each distinct gather-run size
            nidx_regs = {}
            for v in range(1, 9):
                r = nc.gpsimd.alloc_register(f"nidx_{v}")
                nc.gpsimd.reg_mov(r, v * 128)
                nidx_regs[v] = r

            # ---- x0 projection term + bias ----
            x0Tt = x0tp.tile([128, NPAD], dt.float16, tag="x0T")
            nc.sync.dma_start(x0Tt[:], x0T_d[:, :])
            for i in range(NT):
                pp = pps.tile([128, FO], dt.float32, tag="pp")
                nc.tensor.matmul(pp[:], lhsT=x0Tt[:, i * 128:(i + 1) * 128],
                                 rhs=W5t[:, 0:FO],
                                 start=True, stop=False,
                                 skip_group_check=True)
                nc.tensor.matmul(pp[:], lhsT=onescol[:], rhs=biasrow[:],
                                 start=False, stop=True,
                                 skip_group_check=True)
                nc.vector.tensor_copy(out_acc[:, i * FO:(i + 1) * FO], pp[:])

            # ---- diffusion ----
            SUPER = 4                 # gather-groups per pk prefetch DMA
            nt_ = 0 if parts <= 1 else (1 if parts <= 3 else 2)
            nph = {2: 1}.get(parts, 2)
            for t in range(nt_):
                g = graphs[t]
                groups = all_groups[t]
                x1v = x1hbm[t][:, :].rearrange("(i p) f -> p i f", p=128)
                for phase in range(nph if t == 0 else 2):
                    src_dram = x0nm if phase == 0 else x1hbm[t]
                    wslc = W5t[:, (1 + 2 * t + phase) * FO:
                               (2 + 2 * t + phase) * FO]
                    first = True
                    for s0 in range(0, len(groups), SUPER):
                        sgrs = groups[s0:s0 + SUPER]
                        sp0 = sgrs[0]["p0"]
                        spcols = sum(x["qcols"] + x["ccols"] for x in sgrs)
                        pk = pkp.tile([128, pcols_max], dt.int16, tag="pk")
                        nc.gpsimd.dma_start(
                            pk[:, :spcols],
                            pk_d[t][:, sp0:sp0 + spcols])
                        if first and phase == 1:
                            # absorb the x1-store completion waits into a
                            # flexible SWDGE read before the first gather
                            dumm = xTp.tile([1, 64], dt.float16, tag="dumm")
                            nc.gpsimd.dma_start(dumm[:],
                                                src_dram[0:1, 0:64])
                            first = False
                        for gr in sgrs:
                            gch = gr["gch"]
                            qc = gr["qcols"]
                            off = gr["p0"] - sp0
                            st_ = pk[:, off:off + qc].bitcast(dt.float16)
                            # gathers are capped at 1024 idxs (8 chunks of
                            # 128): split the group's chunks into runs and
                            # pipeline one G tile per run
                            runs = []
                            for r0 in range(0, gch, 8):
                                rn = min(8, gch - r0)
                                Gt = gp.tile([128, 8, 128], dt.float16,
                                             tag="G")
                                iq = off + qc + r0 * 8
                                # pre-sync: a tiny Pool op reading pk and
                                # writing G absorbs the gather's sem waits
                                # (the gather struct fits only one wait)
                                nc.gpsimd.tensor_copy(
                                    Gt[0:16, 0, 0:2].bitcast(dt.int16),
                                    pk[0:16, iq:iq + 2])
                                nc.gpsimd.dma_gather(
                                    out_ap=Gt[:, :rn, :],
                                    in_ap=src_dram[:, :],
                                    idxs_ap=pk[:, iq:iq + rn * 8],
                                    num_idxs=rn * 128,
                                    num_idxs_reg=nidx_regs[rn],
                                    elem_size=F,
                                )
                                runs.append(Gt)
                            ntl = len(gr["tiles"])
                            xg = xtp.tile([128, GROUP, F], dt.float16,
                                          tag="xt")
                            cbase = 0
                            for il, i in enumerate(gr["tiles"]):
                                chunks = g["tiles"][i]
                                yp = yps.tile([128, F], dt.float32, tag="yp")
                                nc.tensor.matmul(yp[:], lhsT=onescol[:],
                                                 rhs=zrow[:], start=True,
                                                 stop=False,
                                                 skip_group_check=True)
                                nch = len(chunks)
                                for c, (M_c, o_c, q_c) in enumerate(chunks):
                                    ql = q_c - gr["q0"]
                                    gc = cbase + c
                                    nc.tensor.matmul(
                                        yp[o_c:o_c + M_c, :],
                                        lhsT=st_[:, ql:ql + M_c],
                                        rhs=runs[gc // 8][:, gc % 8, :],
                                        start=False, stop=(c == nch - 1),
                                        tile_position=(0, o_c),
                                        skip_group_check=True)
                                cbase += nch
                                xt_ = xg[:, il, :]
                                nc.vector.tensor_copy(xt_, yp[:])
                                tp = tps.tile([128, 128], dt.float16,
                                              tag="tp")
                                nc.tensor.transpose(tp[:], xt_, ident[:])
                                xT_ = xTp.tile([128, 128], dt.float16,
                                               tag="xT")
                                nc.vector.tensor_copy(xT_[:], tp[:])
                                pp = pps.tile([128, FO], dt.float32,
                                              tag="pp")
                                nc.tensor.matmul(pp[:], lhsT=xT_[:],
                                                 rhs=wslc,
                                                 start=True, stop=True)
                                nc.any.tensor_add(
                                    out_acc[:, i * FO:(i + 1) * FO],
                                    out_acc[:, i * FO:(i + 1) * FO], pp[:])
                            if phase == 0:
                                i0 = gr["tiles"][0]
                                nc.gpsimd.dma_start(
                                    x1v[:, i0:i0 + ntl, :],
                                    xg[:, :ntl, :])

            # ---- store (single DMA; DRAM viewed as [tile, part, fo]) ----
            out_view = out_d[:, :].rearrange("(i p) f -> p i f", p=128)
            nc.gpsimd.dma_start(out_view, out_acc[:, :].rearrange(
                "p (i f) -> p i f", f=FO))
    nc.compile()
    return nc


def kernel(inputs, trans1_idx, trans1_val, trans2_idx, trans2_val,
           weight, bias):
    inputs = np.asarray(inputs, np.float32)
    weight = np.asarray(weight, np.float32)
    bias = np.asarray(bias, np.float32)

    g1 = _prep_graph(np.asarray(trans1_idx), np.asarray(trans1_val))
    g2 = _prep_graph(np.asarray(trans2_idx), np.asarray(trans2_val))

    # folded projection weights: x2 = 2*A@x1 - x0 terms folded into W'
    W = weight.reshape(P, 5, Q)
    w = [W[:, m, :] for m in range(5)]
    wterm = [w[0] - w[2] - w[4], w[1], 2 * w[2], w[3], 2 * w[4]]
    W5 = np.zeros((128, 5 * FO), np.float16)
    for m in range(5):
        for bl in range(4):
            W5[bl * 32:(bl + 1) * 32,
               m * FO + bl * 64:m * FO + (bl + 1) * 64] = wterm[m]
    biasrow = np.tile(bias, 4).reshape(1, FO).astype(np.float16)
    ones_ = np.ones((1, 128), np.float16)
    zrow = np.zeros((1, 128), np.float16)
    ident = np.eye(128, dtype=np.float16)

    grp1 = _make_groups(g1)
    grp2 = _make_groups(g2)
    shared = {"pk1": grp1[1], "pk2": grp2[1],
              "W5": W5, "biasrow": biasrow, "ones_": ones_, "zrow": zrow,
              "ident": ident}

    in_maps = []
    for core in range(NCORES):
        x0 = np.zeros((NPAD, F), np.float16)
        for bl in range(4):
            x0[:N, bl * 32:(bl + 1) * 32] = inputs[4 * core + bl].reshape(N, P)
        in_maps.append({**shared, "x0nm": x0,
                        "x0T": np.ascontiguousarray(x0.T)})

    nc = _build_nc(g1, g2, grp1, grp2)
    res = run_bass_kernel_spmd(nc, in_maps, core_ids=list(range(NCORES)))

    out = np.empty((B, N * Q), np.float32)
    for core in range(NCORES):
        o = res.results[core]["out"]          # [NPAD, FO] f32
        for bl in range(4):
            out[4 * core + bl] = o[:N, bl * 64:(bl + 1) * 64].reshape(N * Q)
    return out


if __name__ == "__main__":
    import reference
    inp = {k: np.asarray(v) for k, v in reference.setup_inputs().items()}
    expected = np.asarray(reference.reference(**inp))
    actual = kernel(**inp)
    rel = np.linalg.norm(actual - expected) / np.linalg.norm(expected)
    print("rel l2 err:", rel)

